# revision 1
# baseline (speedup 1.0000x reference)
"""Segment-mean pooling (segment_sum / counts) + Linear, on 8 TRN2 NeuronCores.

Segment-ownership sharding: the host routes each row to the core that owns
its segment range (core i owns segments [512*i, 512*(i+1))); no collectives.

Per core, segments are split into 4 tiles of 128 (one PSUM bank each), and
the input stream is ordered TILE-MAJOR so each tile's epilogue (scale +
transpose + Linear + store) runs on otherwise-idle engines while the next
tile's rows are still streaming in.  Only the last tile's epilogue is on
the post-stream critical path.

Per tile the rows arrive in three forms, streamed in this order:
  - overflow (rows 24+ per segment, ~4%): 128-row chunks with a DVE-built
    is_equal one-hot [128, 128] stationary.  The first chunk OPENS the
    PSUM bank (start=True); these slow full-width matmuls run early, while
    the PE would otherwise idle waiting on the band DMAs.
  - band A: the first 16 rows of every segment, packed so 4 chunks of 128
    rows form a quad of matmuls against 4 shared block-ones [128, 32]
    stationaries, one per 32-partition column group (the PE runs the 4
    members concurrently).
  - band B: rows 16..24, same quad structure with 8-row slots; the last
    quad CLOSES the bank (stop=True), so the close costs nothing.

The PE's HAM clock gate ignores M=32 quad matmuls, so the PE would sit at
the cold 1.2 GHz clock all kernel; a 12-deep full-width junk warmup burst
plus one junk pulse at each DMA-wait point trips/retains the 2.4 GHz
un-throttle often enough to be worth ~1 us on average (the free-running
4096-cycle HAM window makes individual runs vary +-2 us either way).

Epilogue per tile, software-pipelined into the next tile's band phase:
fence matmul -> DVE cast f32->f16 (per half) -> PE transpose (one ps bank
per half; the bank rule forbids PE-write + DVE-read on one bank) -> DVE
copy to SBUF -> Linear matmuls out[s, j] = sum_h pooled_T[h, s]*W.T[h, j]
-> fence -> DVE scale_by_1/count + bias -> DMA out on the fast queue.
"""

import numpy as np

import concourse.bass as bass
import concourse.mybir as mybir
from concourse.bass_utils import run_bass_kernel_spmd

N_CORES = 8
S_TOTAL = 4096
S_PER = S_TOTAL // N_CORES  # 512 segments per core
N_TILES = 4  # PSUM tiles of 128 segments
H = 256
EPS = np.float32(1e-8)
PAD_IDX = 9999.0  # sentinel relative idx; never matches iota [0, 128)
C = 16  # band-A capacity (rows per segment)
C2 = 8  # band-B capacity (rows 16..24)

KA = 16  # band-A chunks per tile
KB2 = 8  # band-B chunks per tile

# cf16 const layout (f16 columns)
ONES_OFF = 0  # 6 patterns x 32 (A g0..g3, B h0..h1)
IDENT_OFF = 192
IOTA_OFF = 320
WT_OFF = 448  # 2 x 256
CF16_W = 960
# cf32 const layout (f32 columns): invc[4], bb[256], ovidx[OVK]
BB_OFF = 4
OVIDX_OFF = 260

_graph_cache: dict = {}


def _build(ovks: tuple) -> "bass.Bass":
    """ovks[t] = number of overflow chunks for tile t (>=1, SPMD-shared)."""
    f16 = mybir.dt.float16
    f32 = mybir.dt.float32
    OVK = sum(ovks)
    NREST = 4 * KB2 + OVK
    roff = [0]  # per-tile xrest base: [ov chunks..., B chunks...]
    for t in range(N_TILES):
        roff.append(roff[-1] + ovks[t] + KB2)

    nc = bass.Bass()

    xb_d = nc.declare_dram_parameter("xb", [128, 64, H], f16, isOutput=False)
    xr_d = nc.declare_dram_parameter("xr", [128, NREST, H], f16, isOutput=False)
    cf16_d = nc.declare_dram_parameter("cf16", [128, CF16_W], f16, isOutput=False)
    cf32_d = nc.declare_dram_parameter(
        "cf32", [128, OVIDX_OFF + OVK], f32, isOutput=False
    )
    out_d = nc.declare_dram_parameter("out", [S_PER, H], f32, isOutput=True)

    from contextlib import ExitStack

    with ExitStack() as ctx:
        xbb = ctx.enter_context(nc.sbuf_tensor("xbb", [128, 64, H], f16))
        xrr = ctx.enter_context(nc.sbuf_tensor("xrr", [128, NREST, H], f16))
        cf16 = ctx.enter_context(nc.sbuf_tensor("cf16s", [128, CF16_W], f16))
        cf32 = ctx.enter_context(
            nc.sbuf_tensor("cf32s", [128, OVIDX_OFF + OVK], f32)
        )
        oh = ctx.enter_context(nc.sbuf_tensor("oh", [128, OVK, 128], f16))
        junk = ctx.enter_context(nc.sbuf_tensor("junk", [128, 512], f16))
        pool = ctx.enter_context(nc.sbuf_tensor("pool", [128, N_TILES, H], f16))
        sums2 = ctx.enter_context(nc.sbuf_tensor("sums2", [128, 8, 128], f16))
        out_sb = ctx.enter_context(nc.sbuf_tensor("outsb", [128, N_TILES, H], f32))
        ps_s = [
            ctx.enter_context(nc.psum_tensor(f"ps_s{t}", [128, 512], f32))
            for t in range(N_TILES)
        ]
        ps_tab = [
            ctx.enter_context(nc.psum_tensor(f"ps_tab{i}", [128, 1024], f16))
            for i in range(2)
        ]
        ps_x = ctx.enter_context(nc.psum_tensor("ps_x", [128, 512], f32))

        rsem = [ctx.enter_context(nc.semaphore(f"rs{t}")) for t in range(4)]
        a1sem = [ctx.enter_context(nc.semaphore(f"a1s{t}")) for t in range(4)]
        a2sem = [ctx.enter_context(nc.semaphore(f"a2s{t}")) for t in range(4)]
        a2bsem = ctx.enter_context(nc.semaphore("a2bs"))
        c16sem = ctx.enter_context(nc.semaphore("c16sem"))
        c32sem = ctx.enter_context(nc.semaphore("c32sem"))
        ohsem = ctx.enter_context(nc.semaphore("ohsem"))
        mmsem = ctx.enter_context(nc.semaphore("mmsem"))
        castsem = ctx.enter_context(nc.semaphore("castsem"))
        trsem = ctx.enter_context(nc.semaphore("trsem"))
        cp2sem = ctx.enter_context(nc.semaphore("cp2sem"))
        mmesem = ctx.enter_context(nc.semaphore("mmesem"))
        oesem = ctx.enter_context(nc.semaphore("oesem"))
        dmasem = ctx.enter_context(nc.semaphore("dmasem"))
        block = ctx.enter_context(nc.Block())

        ident = cf16[:, IDENT_OFF : IDENT_OFF + 128]
        iota = cf16[:, IOTA_OFF : IOTA_OFF + 128]
        zl = cf16[0:1, 0:128]  # junk; fence targets ps_x which is never read
        zr = cf16[0:1, 0:8]

        @block.scalar
        def _(scalar):
            # x DMAs ride this one ring / hardware-dynamic queue: its 16
            # engines hit ~415 GB/s with 4-6 KB descriptors.  Consts go on
            # the sync queue so they don't delay the stream (the sync queue
            # is only safe for EARLY transfers; late ones starve the shared
            # DMA engines).
            for t in range(N_TILES):
                scalar.dma_start(
                    out=xrr[:, roff[t] : roff[t + 1], :],
                    in_=xr_d[:, roff[t] : roff[t + 1], :],
                ).then_inc(rsem[t], 16)
                scalar.dma_start(
                    out=xbb[:, 16 * t : 16 * t + 8, :],
                    in_=xb_d[:, 16 * t : 16 * t + 8, :],
                ).then_inc(a1sem[t], 16)
                if t < 3:
                    scalar.dma_start(
                        out=xbb[:, 16 * t + 8 : 16 * t + 16, :],
                        in_=xb_d[:, 16 * t + 8 : 16 * t + 16, :],
                    ).then_inc(a2sem[t], 16)
                else:
                    # the last group gates the tail: split it so the g2
                    # quad runs while the final 0.26 MB still streams
                    scalar.dma_start(
                        out=xbb[:, 56:60, :], in_=xb_d[:, 56:60, :]
                    ).then_inc(a2sem[t], 16)
                    scalar.dma_start(
                        out=xbb[:, 60:64, :], in_=xb_d[:, 60:64, :]
                    ).then_inc(a2bsem, 16)
            # outputs ride the same fast queue; descriptors enter the wire
            # after all input descriptors, so the stream is unaffected
            for t in range(N_TILES):
                scalar.wait_ge(oesem, t + 1)
                scalar.dma_start(
                    out=out_d[128 * t : 128 * (t + 1), :], in_=out_sb[:, t, :]
                ).then_inc(dmasem, 16)
            for t in range(N_TILES):
                for sem in (rsem, a1sem, a2sem):
                    scalar.wait_ge(sem[t], 16)
            scalar.wait_ge(a2bsem, 16)
            scalar.wait_ge(dmasem, 16 * N_TILES)

        @block.sync
        def _(sync):
            sync.dma_start(out=cf16[:, :], in_=cf16_d[:, :]).then_inc(c16sem, 16)
            sync.dma_start(out=cf32[:, :], in_=cf32_d[:, :]).then_inc(c32sem, 16)
            sync.wait_ge(c16sem, 16)
            sync.wait_ge(c32sem, 16)

        @block.vector
        def _(vector):
            # overflow one-hots, built up-front while the stream runs
            vector.wait_ge(c16sem, 16)
            vector.wait_ge(c32sem, 16)
            for j in range(OVK):
                vector.tensor_scalar(
                    out=oh[:, j, :],
                    in0=iota,
                    scalar1=cf32[:, OVIDX_OFF + j : OVIDX_OFF + j + 1],
                    scalar2=None,
                    op0=mybir.AluOpType.is_equal,
                ).then_inc(ohsem, 1)
            # per-tile epilogue stages, half-pipelined
            for t in range(N_TILES):
                vector.wait_ge(mmsem, t + 1)
                for hb in range(2):
                    vector.tensor_copy(
                        out=pool[:, t, 128 * hb : 128 * (hb + 1)],
                        in_=ps_s[t][:, 128 * hb : 128 * (hb + 1)],
                    ).then_inc(castsem, 1)
                vector.wait_ge(trsem, t + 1)
                for hb in range(2):
                    vector.tensor_copy(
                        out=sums2[:, 2 * t + hb, :],
                        in_=ps_tab[hb][:, 0:128],
                    ).then_inc(cp2sem, 1)
                vector.wait_ge(mmesem, t + 1)
                vector.scalar_tensor_tensor(
                    out=out_sb[:, t, :],
                    in0=ps_s[t][:, 0:H],
                    scalar=cf32[:, t : t + 1],
                    in1=cf32[:, BB_OFF : BB_OFF + H],
                    op0=mybir.AluOpType.mult,
                    op1=mybir.AluOpType.add,
                ).then_inc(oesem, 1)

        @block.tensor
        def _(tensor):
            def tr_pair(t):
                # transpose pooled halves; one bank per half (the bank rule
                # forbids PE-write + DVE-read on one bank), one fence for
                # the pair
                for hb in range(2):
                    tensor.wait_ge(castsem, 2 * t + hb + 1)
                    if t >= 1 and hb == 0:
                        # previous tile's copies of both banks must be done
                        tensor.wait_ge(cp2sem, 2 * t)
                    for a in range(4):
                        tensor.transpose(
                            ps_tab[hb][32 * a : 32 * a + 32, 0:128],
                            pool[:, t, 128 * hb + 32 * a : 128 * hb + 32 * a + 32],
                            ident,
                            tile_position=(0, 32 * a),
                        )
                tensor.matmul(
                    ps_x[:, 0:8], zl, zr, start=True, stop=True,
                    skip_group_check=True,
                ).then_inc(trsem, 1)

            def lin_pair(t):
                # Linear: out[s, j] = sum_h pooled_T[h, s] * W.T[h, j]
                for hb in range(2):
                    tensor.wait_ge(cp2sem, 2 * t + hb + 1)
                    tensor.matmul(
                        ps_s[t][:, 0:H],
                        sums2[:, 2 * t + hb, :],
                        cf16[:, WT_OFF + 256 * hb : WT_OFF + 256 * (hb + 1)],
                        start=(hb == 0),
                        stop=(hb == 1),
                        skip_group_check=True,
                    )
                tensor.matmul(
                    ps_x[:, 0:8], zl, zr, start=True, stop=True,
                    skip_group_check=True,
                ).then_inc(mmesem, 1)

            def pulse(n=1):
                # full-width junk matmul into the never-read ps_x bank.
                # M=32 band quads don't register as PE activity for the HAM
                # clock gate; only full-width work does.  The warmup burst
                # trips the un-throttle (1.2 -> 2.4 GHz) and a pulse at
                # every DMA-wait point keeps the MID window from
                # re-throttling.  Values are junk; ps_x is never read.
                for _ in range(n):
                    tensor.matmul(
                        ps_x[:, 0:256], junk[:, 0:128], junk[:, 0:256],
                        start=True, stop=True, skip_group_check=True,
                    )

            pulse(12)  # ~4 us sustained: covers the HAM SHORT window
            tensor.wait_ge(c16sem, 16)
            ohcum = 0
            for t in range(N_TILES):
                pulse()
                # overflow one-hot chunks; the first opens the bank.  These
                # slow full-width matmuls run while the PE would otherwise
                # idle waiting on the A-band DMAs.
                tensor.wait_ge(ohsem, ohcum + ovks[t])
                tensor.wait_ge(rsem[t], 16)
                for jj in range(ovks[t]):
                    tensor.matmul(
                        ps_s[t][:, 0:H],
                        oh[:, ohcum + jj, :],
                        xrr[:, roff[t] + jj, :],
                        start=(jj == 0),
                        stop=False,
                        skip_group_check=True,
                    )
                ohcum += ovks[t]
                # band A g0-g1, then band B (data arrived with rsem), then
                # A g2-g3 last so the close rides the last-arriving quad
                pulse()
                tensor.wait_ge(a1sem[t], 16)
                for g in range(2):
                    for m in range(4):
                        tensor.matmul(
                            ps_s[t][32 * m : 32 * m + 32, 0:H],
                            cf16[:, 32 * g : 32 * g + 32],
                            xbb[:, 16 * t + 4 * g + m, :],
                            start=False,
                            stop=False,
                            skip_group_check=True,
                            tile_position=(0, 32 * m),
                        )
                for h in range(2):
                    for m in range(4):
                        tensor.matmul(
                            ps_s[t][32 * m : 32 * m + 32, 0:H],
                            cf16[:, 128 + 32 * h : 128 + 32 * h + 32],
                            xrr[:, roff[t] + ovks[t] + 4 * h + m, :],
                            start=False,
                            stop=False,
                            skip_group_check=True,
                            tile_position=(0, 32 * m),
                        )
                # previous tile's transposes fill the a2 DMA-wait slack
                if t >= 1:
                    tr_pair(t - 1)
                pulse()
                tensor.wait_ge(a2sem[t], 16)
                for g in range(2, 4):
                    if t == 3 and g == 3:
                        tensor.wait_ge(a2bsem, 16)
                    for m in range(4):
                        tensor.matmul(
                            ps_s[t][32 * m : 32 * m + 32, 0:H],
                            cf16[:, 32 * g : 32 * g + 32],
                            xbb[:, 16 * t + 4 * g + m, :],
                            start=False,
                            stop=(g == 3),
                            skip_group_check=True,
                            tile_position=(0, 32 * m),
                        )
                # fence: hand the bank to DVE only after writes drain
                tensor.matmul(
                    ps_x[:, 0:8], zl, zr, start=True, stop=True,
                    skip_group_check=True,
                ).then_inc(mmsem, 1)
                # previous tile's Linear rides behind this tile's close
                if t >= 1:
                    lin_pair(t - 1)
            tr_pair(N_TILES - 1)
            lin_pair(N_TILES - 1)

    return nc


def kernel(x, dst_idx, dst_size, W, b):
    x = np.asarray(x)
    idx = np.asarray(dst_idx).astype(np.int64)
    W = np.asarray(W, dtype=np.float32)
    b = np.asarray(b, dtype=np.float32)
    S = int(dst_size)
    assert S == S_TOTAL and x.shape[1] == H

    counts = np.bincount(idx, minlength=S).astype(np.float32)
    inv = np.float32(1.0) / (counts + EPS)

    order = np.argsort(idx, kind="stable")
    sidx = idx[order]
    bounds = np.searchsorted(sidx, np.arange(0, S + 1, S_PER))

    x16 = x.astype(np.float16)

    bands, rests_b, ovs, ovsegs = [], [], [], []
    for i in range(N_CORES):
        lo_i, hi_i = bounds[i], bounds[i + 1]
        n_i = hi_i - lo_i
        li = (sidx[lo_i:hi_i] - S_PER * i).astype(np.int64)
        rows = order[lo_i:hi_i]
        starts = np.searchsorted(li, np.arange(S_PER + 1))
        rank = np.arange(n_i) - starts[li]
        t_, u = li // 128, li % 128
        m_, w = u // 32, u % 32
        # band A
        bm = rank < C
        cA = 16 * t_[bm] + 4 * (w[bm] // 8) + m_[bm]
        rA = 16 * (w[bm] % 8) + rank[bm]
        xband = np.zeros((128, 64, H), dtype=np.float16)
        xband[rA, cA] = x16[rows[bm]]
        bands.append(xband)
        # band B
        bm2 = (rank >= C) & (rank < C + C2)
        cB = 8 * t_[bm2] + 4 * (w[bm2] // 16) + m_[bm2]
        rB = 8 * (w[bm2] % 16) + (rank[bm2] - C)
        rests_b.append((cB, rB, rows[bm2]))
        # overflow, per tile
        om = rank >= C + C2
        ovs.append(rows[om])
        ovsegs.append((t_[om], u[om]))

    # SPMD-shared overflow chunk counts per tile
    ovks = []
    for t in range(N_TILES):
        mx = 1
        for i in range(N_CORES):
            nt = int(np.sum(ovsegs[i][0] == t))
            mx = max(mx, -(-nt // 128))
        ovks.append(mx)
    ovks = tuple(ovks)
    OVK = sum(ovks)
    NREST = 4 * KB2 + OVK
    roff = [0]
    for t in range(N_TILES):
        roff.append(roff[-1] + ovks[t] + KB2)

    key = ovks
    nc = _graph_cache.get(key)
    if nc is None:
        nc = _build(ovks)
        _graph_cache[key] = nc

    # shared f16 consts
    cf16_np = np.zeros((128, CF16_W), dtype=np.float16)
    r = np.arange(128)
    for g in range(4):  # band A stationaries
        cf16_np[r, ONES_OFF + 32 * g + 8 * g + r // C] = 1.0
    for h in range(2):  # band B stationaries
        cf16_np[r, ONES_OFF + 128 + 32 * h + 16 * h + r // C2] = 1.0
    cf16_np[r, IDENT_OFF + r] = 1.0
    cf16_np[:, IOTA_OFF : IOTA_OFF + 128] = np.arange(128, dtype=np.float16)
    for hb in range(2):
        # wt[p, 256*hb + j] = W[j, 128*hb + p]
        cf16_np[:, WT_OFF + 256 * hb : WT_OFF + 256 * (hb + 1)] = (
            W[:, 128 * hb : 128 * (hb + 1)].T.astype(np.float16)
        )

    in_maps = []
    for i in range(N_CORES):
        xr_np = np.zeros((128, NREST, H), dtype=np.float16)
        cB, rB, rowsB = rests_b[i]
        # band B chunks: tile t's chunk k lives at xrest slot roff[t]+ovks[t]+k
        tB = cB // 8
        xr_np[rB, np.array(roff)[tB] + ovks_arr(ovks)[tB] + (cB - 8 * tB)] = x16[
            rowsB
        ]
        # overflow chunks at the front of each tile's xrest span
        tv, uv = ovsegs[i]
        ovrows = ovs[i]
        cf32_np = np.zeros((128, OVIDX_OFF + OVK), dtype=np.float32)
        cf32_np[:, OVIDX_OFF:] = PAD_IDX
        for t in range(N_TILES):
            sel = tv == t
            rows_t = ovrows[sel]
            u_t = uv[sel]
            n_t = len(rows_t)
            ro = np.arange(n_t)
            xr_np[ro % 128, roff[t] + ro // 128] = x16[rows_t]
            cf32_np[ro % 128, OVIDX_OFF + sum(ovks[:t]) + ro // 128] = u_t
        cf32_np[:, 0:4] = inv[S_PER * i : S_PER * (i + 1)].reshape(4, 128).T
        cf32_np[:, BB_OFF : BB_OFF + H] = b
        in_maps.append(
            {
                "xb": bands[i],
                "xr": xr_np,
                "cf16": cf16_np,
                "cf32": cf32_np,
            }
        )

    res = run_bass_kernel_spmd(nc, in_maps, core_ids=list(range(N_CORES)))
    return np.concatenate([res.results[i]["out"] for i in range(N_CORES)], axis=0)


def ovks_arr(ovks):
    return np.array(ovks)

